# revision 53
# baseline (speedup 1.0000x reference)
"""MoE genre-gate kernel for 8 Trainium2 NeuronCores.

Strategy (expert-parallel with token dispatch, per sharding hint):
  - Routing (RMSNorm -> word+genre gate -> softmax -> top-2) is computed on
    host in float64: it is 0.03% of the FLOPs and produces the data-dependent
    dispatch tables (the stand-in for all-to-all).
  - The 8192 (token, expert) pairs are grouped per expert and packed into
    per-core slots by a capacity-profile search (_partition): it finds slot
    capacities CS (identical on every core = SPMD) minimizing total streamed
    columns, then bin-packs expert token chunks into the (core, slot) grid.
    For the seed-0 data this yields CS=[416, 384, 288] (1088 cols/core vs
    the 1024 ideal).
  - Each core runs a dense 3-stage MLP over its expert slots in bf16
    (full PE stream rate, FWL-fast weight loads, half the HBM traffic of
    fp32; ~3e-3 end-to-end rel err) with fp32 PSUM accumulation.  With zero
    in-MLP biases the host pre-scales each token row by its combine weight
    cw >= 0 (commutes with relu), so padding rows are exactly zero and
    stage 3 runs weight-stationary; a general biased fallback path is kept.
  - DMA: weight streams are spread over the three DMA-capable queues
    (w1+xt-tail on gpsimd, w2 on sync, w3+xt on scalar) with 8-deep tile
    pools; slot-0's first 8 w1 tiles (sync) and 8 w2 tiles (scalar) are
    prefetched at the program head so the PE never starves during the
    DMA-ring ramp-up.  10 junk matmuls warm the HAM clock-gate to 2.4 GHz
    while the first loads land, with no idle gap into the real stream.
  - Stage 3 of slot s-1 is software-interleaved with stage 1 of slot s to
    keep the PE busy across stage boundaries; stage-3 outputs drain via the
    idle DVE (vector) engine as bf16.
  - Host scatter-adds the per-pair outputs back to [B,S,H] and adds the
    (cw @ b3) bias term.

Measured: 339.9us (baseline) -> ~256us; PE streams at its 1-col/cycle
roofline (232us for 1088 cols x 512 matmul-cycles/col) inside the body.

Hardcoded problem shape: B=2, S=2048, H=1024, G=256, E=8, M=2048, top-2.
"""

import numpy as np

import concourse.bass as bass
import concourse.tile as tile
from concourse import mybir
from concourse.bass_utils import run_bass_kernel_spmd

TOP_K = 2
EPS = 1e-6
N_CORES = 8
H = 1024
M = 2048
KH, KM = H // 128, M // 128
F32R = mybir.dt.float32r
F32 = mybir.dt.float32
BF16 = mybir.dt.bfloat16
MM_DT = BF16          # matmul dtype (bf16: full PE rate, FWL 2x weight loads, half DMA)


# ---------------------------------------------------------------------------
# walrus in this container accepts only ONE sync-wait command per
# instruction; Tile emits up to ~10.  Split extras onto standalone NoOps on
# the same engine, inserted immediately before the instruction, which
# preserves per-engine program order and therefore semantics.
_ctr = [0]


def _legalize_waits(nc, max_waits=1):
    for f in nc.m.functions:
        for blk in f.blocks:
            out = []
            for inst in blk.instructions:
                si = inst.sync_info
                if si is not None and len(si.on_wait) > max_waits:
                    waits = list(si.on_wait)
                    extra, keep = waits[:-max_waits], waits[-max_waits:]
                    for w in extra:
                        _ctr[0] += 1
                        out.append(mybir.InstNoOp(
                            name=f"waitsplit-{_ctr[0]}",
                            engine=inst.engine, ins=[], outs=[],
                            sync_info=mybir.SyncInfo(on_wait=[w], on_update=[]),
                        ))
                    inst.sync_info = mybir.SyncInfo(
                        on_wait=keep, on_update=list(si.on_update))
                out.append(inst)
            blk.instructions = out


# ---------------------------------------------------------------------------
def _route(x2d, genre_embed, rms_w, wg_W, wg_b, gg_W, gg_b, B, S):
    """Host gating in float64. Returns combine weights [T, E] (zero outside
    top-2)."""
    xd = x2d.astype(np.float64)
    var = np.mean(xd * xd, axis=-1, keepdims=True)
    xn = rms_w.astype(np.float64) * (xd / np.sqrt(var + EPS))
    gate = xn @ wg_W.astype(np.float64) + wg_b.astype(np.float64)
    gg = genre_embed.astype(np.float64)[:, 0, :] @ gg_W.astype(np.float64) \
        + gg_b.astype(np.float64)                       # [B, E]
    gate = gate.reshape(B, S, -1) + gg[:, None, :]
    gate = gate.reshape(B * S, -1)
    gate -= gate.max(axis=-1, keepdims=True)
    p = np.exp(gate)
    p /= p.sum(axis=-1, keepdims=True)
    top2 = np.argsort(-p, axis=-1)[:, :TOP_K]
    cw = np.zeros_like(p)
    rows = np.arange(p.shape[0])[:, None]
    cw[rows, top2] = p[rows, top2]
    return cw.astype(np.float32)


def _fit(counts, CS):
    """Cover counts with bins (N_CORES bins of capacity CS[j] per class j);
    each bin holds one contiguous chunk of one expert.  Returns n[e][j] bin
    usage, or None if infeasible.  DFS over experts (desc count) with a
    global waste budget and failed-state memo."""
    E, K = len(counts), len(CS)
    slack = N_CORES * sum(CS) - sum(counts)
    if slack < 0:
        return None
    order = sorted(range(E), key=lambda e: -counts[e])
    failed = set()

    def combos(need, avail):
        """Minimal bin combos covering `need`, sorted by waste."""
        out = []

        def rec(j, left, cur):
            if left <= 0:
                out.append(tuple(cur) + (0,) * (K - len(cur)))
                return
            if j == K:
                return
            if sum(avail[i] * CS[i] for i in range(j, K)) < left:
                return
            for n in range(min(avail[j], -(-left // CS[j])), -1, -1):
                rec(j + 1, left - n * CS[j], cur + [n])

        rec(0, need, [])
        mins = [c for c in out
                if all(c[j] == 0 or sum(ci * CS[i] for i, ci in enumerate(c))
                       - CS[j] < need for j in range(K))]
        mins.sort(key=lambda c: sum(ci * CS[i] for i, ci in enumerate(c)))
        return mins

    def dfs(ei, avail, budget):
        if ei == E:
            return []
        key = (ei, tuple(avail))
        if key in failed:
            return None
        e = order[ei]
        for c in combos(counts[e], avail):
            w = sum(ci * CS[i] for i, ci in enumerate(c)) - counts[e]
            if w > budget:
                break
            rest = dfs(ei + 1, [a - ci for a, ci in zip(avail, c)],
                       budget - w)
            if rest is not None:
                return [c] + rest
        failed.add(key)
        return None

    sol = dfs(0, [N_CORES] * K, slack)
    if sol is None:
        return None
    n = [[0] * K for _ in range(E)]
    for ei, c in enumerate(sol):
        n[order[ei]] = list(c)
    return n


def _partition(counts):
    """Choose per-core slot capacities CS and assign expert token chunks to
    (core, slot) bins, minimizing total streamed columns (sum CS) with a
    small penalty per extra slot class (extra weight DMA + ldweights).

    Returns (CS, slots): slots[core][j] = (expert, lo, hi) or None."""
    E = len(counts)
    cand = []
    caps = list(range(512, 63, -32))
    import itertools
    for K in (2, 3, 4):
        for prof in itertools.combinations_with_replacement(caps, K):
            prof = tuple(sorted(prof, reverse=True))
            score = (sum(prof) + 64 * (K - 2), K, -min(prof))
            cand.append((score, prof))
    cand.sort()
    seen = set()
    best = None
    for score, prof in cand:
        if prof in seen:
            continue
        seen.add(prof)
        if best is not None and score[0] >= best[0][0]:
            break
        n = _fit(counts, list(prof))
        if n is not None:
            best = (score, prof, n)
            break
    assert best is not None, "no feasible capacity profile"
    _, CS, n = best
    K = len(CS)
    # materialize chunks: fill each expert's bins largest-class-first
    slots = [[None] * K for _ in range(N_CORES)]
    next_core = [0] * K
    for e in range(E):
        lo = 0
        for j in range(K):
            for _ in range(n[e][j]):
                hi = min(lo + CS[j], counts[e])
                slots[next_core[j]][j] = (e, lo, hi)
                next_core[j] += 1
                lo = hi
        assert lo == counts[e]
    return list(CS), slots


def _token_chunks(C):
    """Split C into matmul moving-dim chunks, each <=512 (PSUM bank) and as
    equal as possible (>=256 keeps float32r at full rate)."""
    assert C % 2 == 0
    n = -(-C // 512)
    h = C // 2
    base, rem = divmod(h, n)
    sizes = [2 * (base + (1 if i < rem else 0)) for i in range(n)]
    assert sum(sizes) == C and all(s <= 512 and s % 2 == 0 for s in sizes)
    return sizes


# ---------------------------------------------------------------------------
def _build_program(CS, prescaled=False, legalize=True):
    """Emit the SPMD Bass program; CS = per-slot-class capacities.

    prescaled=True (valid when b1==b2==0): host pre-scales x rows by cw
    (cw>=0 commutes with relu), so no bias/cw tiles are needed and stage 3
    runs weight-stationary with output layout [H, C] (y transposed)."""
    S = len(CS)
    nc = bass.Bass()
    xt_d = [nc.dram_tensor(f"XT{s}", [128, KH * CS[s]], MM_DT, kind="ExternalInput") for s in range(S)]
    w1_d = [nc.dram_tensor(f"W1{s}", [KM, 128, H], MM_DT, kind="ExternalInput") for s in range(S)]
    w2_d = [nc.dram_tensor(f"W2{s}", [KM, 128, M], MM_DT, kind="ExternalInput") for s in range(S)]
    if prescaled:
        w3_d = [nc.dram_tensor(f"W3{s}", [KH, 128, M], MM_DT, kind="ExternalInput") for s in range(S)]
        y_d = [nc.dram_tensor(f"Y{s}", [H, CS[s]], BF16, kind="ExternalOutput") for s in range(S)]
    else:
        w3_d = [nc.dram_tensor(f"W3{s}", [M, H], MM_DT, kind="ExternalInput") for s in range(S)]
        y_d = [nc.dram_tensor(f"Y{s}", [CS[s], H], F32, kind="ExternalOutput") for s in range(S)]
        b1_d = [nc.dram_tensor(f"B1{s}", [M], F32, kind="ExternalInput") for s in range(S)]
        b2_d = [nc.dram_tensor(f"B2{s}", [M], F32, kind="ExternalInput") for s in range(S)]
        cw_d = [nc.dram_tensor(f"CW{s}", [CS[s]], F32, kind="ExternalInput") for s in range(S)]

    HB = 512 if len(CS) >= 3 else 256

    with tile.TileContext(nc) as tc:
        with (
            tc.tile_pool(name="xt", bufs=2) as p_xt,
            tc.tile_pool(name="w", bufs=8) as p_w,
            tc.tile_pool(name="h", bufs=1) as p_h,
            tc.tile_pool(name="bias", bufs=1) as p_b,
            tc.tile_pool(name="y", bufs=4) as p_y,
            tc.tile_pool(name="ps", bufs=8, space="PSUM") as p_ps,
        ):
            p_w1 = p_w2 = p_w3 = p_w
            p_h1 = p_h2 = p_h
            st = [dict() for _ in range(S)]   # per-slot tiles/geometry

            def emit_loads(s):
                C = CS[s]
                v = st[s]
                v["tcs"] = _token_chunks(C)
                v["tco"] = np.cumsum([0] + v["tcs"]).tolist()
                v["tts"] = [(i * 128, min(128, C - i * 128)) for i in range(-(-C // 128))]
                xt_all = p_xt.tile([128, KH * C], MM_DT, tag="xt", name=f"xt_{s}")
                half = KH * C // 2
                nc.scalar.dma_start(out=xt_all[:, :half], in_=xt_d[s][:, :half])
                nc.scalar.dma_start(out=xt_all[:, half:], in_=xt_d[s][:, half:])
                v["xt_all"] = xt_all
                if prescaled:
                    v["h1"] = p_h1.tile([128, KM * C], MM_DT, tag="h1", name=f"h1_{s}")
                    v["h2"] = p_h2.tile([128, KM * C], MM_DT, tag="h2", name=f"h2_{s}")
                    return
                v["b1t"] = [p_b.tile([128, 1], F32, tag=f"b1_{s}_{m}", name=f"b1t_{s}_{m}") for m in range(KM)]
                v["b2t"] = [p_b.tile([128, 1], F32, tag=f"b2_{s}_{m}", name=f"b2t_{s}_{m}") for m in range(KM)]
                for m in range(KM):
                    nc.scalar.dma_start(out=v["b1t"][m][:], in_=b1_d[s][m * 128:(m + 1) * 128].rearrange("(p one) -> p one", one=1))
                    nc.scalar.dma_start(out=v["b2t"][m][:], in_=b2_d[s][m * 128:(m + 1) * 128].rearrange("(p one) -> p one", one=1))
                v["cwt"] = []
                for t, (t0, tn) in enumerate(v["tts"]):
                    v["cwt"].append(p_b.tile([tn, 1], F32, tag=f"cw_{s}_{t}", name=f"cwt_{s}_{t}"))
                    nc.scalar.dma_start(out=v["cwt"][t][:], in_=cw_d[s][t0:t0 + tn].rearrange("(p one) -> p one", one=1))
                v["h1"] = [p_h1.tile([128, C], MM_DT, tag=f"h1_{m}", name=f"h1_{s}_{m}") for m in range(KM)]
                v["h2"] = [p_h2.tile([128, C], MM_DT, tag=f"h2_{m}", name=f"h2_{s}_{m}") for m in range(KM)]

            pre_w1 = {}

            def h1s(v, s, m, a, b):
                if prescaled:
                    return v["h1"][:, m * CS[s] + a:m * CS[s] + b]
                return v["h1"][m][:, a:b]

            def h2s(v, s, m, a, b):
                if prescaled:
                    return v["h2"][:, m * CS[s] + a:m * CS[s] + b]
                return v["h2"][m][:, a:b]

            def st1_group(s, m):
                v = st[s]
                if s == 0 and m in pre_w1:
                    w1t = pre_w1[m]
                else:
                    w1t = p_w1.tile([128, H], MM_DT, tag="w1", name=f"w1t_{s}_{m}")
                    nc.gpsimd.dma_start(out=w1t[:], in_=w1_d[s][m])
                pss = [p_ps.tile([128, tcz], F32, tag="ps", name=f"ps1_{s}_{m}_{ci}")
                       for ci, tcz in enumerate(v["tcs"])]
                C = CS[s]
                for k in range(KH):
                    for ci in range(len(v["tcs"])):
                        nc.tensor.matmul(
                            pss[ci][:], w1t[:, k * 128:(k + 1) * 128],
                            v["xt_all"][:, k * C + v["tco"][ci]:k * C + v["tco"][ci + 1]],
                            start=(k == 0), stop=(k == KH - 1))
                for ci in range(len(v["tcs"])):
                    nc.scalar.activation(
                        h1s(v, s, m, v["tco"][ci], v["tco"][ci + 1]), pss[ci][:],
                        mybir.ActivationFunctionType.Relu,
                        **({} if prescaled else {"bias": v["b1t"][m][:, 0:1]}))

            pre_w2 = {}

            def st2_group(s, m):
                v = st[s]
                if s == 0 and m in pre_w2:
                    w2t = pre_w2[m]
                else:
                    w2t = p_w2.tile([128, M], MM_DT, tag="w2", name=f"w2t_{s}_{m}")
                    nc.sync.dma_start(out=w2t[:], in_=w2_d[s][m])
                pss = [p_ps.tile([128, tcz], F32, tag="ps", name=f"ps2_{s}_{m}_{ci}")
                       for ci, tcz in enumerate(v["tcs"])]
                for k in range(KM):
                    for ci in range(len(v["tcs"])):
                        nc.tensor.matmul(
                            pss[ci][:], w2t[:, k * 128:(k + 1) * 128],
                            h1s(v, s, k, v["tco"][ci], v["tco"][ci + 1]),
                            start=(k == 0), stop=(k == KM - 1))
                for ci in range(len(v["tcs"])):
                    nc.scalar.activation(
                        h2s(v, s, m, v["tco"][ci], v["tco"][ci + 1]), pss[ci][:],
                        mybir.ActivationFunctionType.Relu,
                        **({} if prescaled else {"bias": v["b2t"][m][:, 0:1]}))

            def emit_w3(s, hb):
                w3t = [p_w3.tile([128, HB], MM_DT, tag=f"w3_{k % 4}", name=f"w3t_{s}_{hb}_{k}") for k in range(KM)]
                for k in range(KM):
                    nc.gpsimd.dma_start(
                        out=w3t[k][:],
                        in_=w3_d[s][k * 128:(k + 1) * 128, hb * HB:(hb + 1) * HB])
                st[s][f"w3_{hb}"] = w3t

            def st3_group(s, hb, t):
                v = st[s]
                t0, tn = v["tts"][t]
                w3t = v[f"w3_{hb}"]
                ps = p_ps.tile([tn, HB], F32, tag="ps", name=f"ps3_{s}_{hb}_{t}")
                for k in range(KM):
                    nc.tensor.matmul(
                        ps[:], h2s(v, s, k, t0, t0 + tn), w3t[k][:],
                        start=(k == 0), stop=(k == KM - 1))
                yt = p_y.tile([tn, HB], F32, tag="y", name=f"yt_{s}_{hb}_{t}")
                nc.scalar.activation(
                    yt[:], ps[:], mybir.ActivationFunctionType.Copy,
                    scale=v["cwt"][t][:, 0:1])
                nc.scalar.dma_start(
                    out=y_d[s][t0:t0 + tn, hb * HB:(hb + 1) * HB],
                    in_=yt[:])

            def st3_group_ws(s, hm):
                """Weight-stationary stage 3 (prescaled mode): out y_T[h, tok]."""
                v = st[s]
                w3t = p_w3.tile([128, M], MM_DT, tag="w3ws", name=f"w3ws_{s}_{hm}")
                nc.scalar.dma_start(out=w3t[:], in_=w3_d[s][hm])
                pss = [p_ps.tile([128, tcz], F32, tag="ps", name=f"ps3_{s}_{hm}_{ci}")
                       for ci, tcz in enumerate(v["tcs"])]
                for k in range(KM):
                    for ci in range(len(v["tcs"])):
                        nc.tensor.matmul(
                            pss[ci][:], w3t[:, k * 128:(k + 1) * 128],
                            h2s(v, s, k, v["tco"][ci], v["tco"][ci + 1]),
                            start=(k == 0), stop=(k == KM - 1))
                for ci, tcz in enumerate(v["tcs"]):
                    yt = p_y.tile([128, tcz], BF16, tag="y", name=f"yt_{s}_{hm}_{ci}")
                    nc.vector.tensor_scalar_mul(yt[:], pss[ci][:], 1.0)
                    nc.gpsimd.dma_start(
                        out=y_d[s][hm * 128:(hm + 1) * 128, v["tco"][ci]:v["tco"][ci + 1]],
                        in_=yt[:])

            def st3_emitters(s):
                if prescaled:
                    return [lambda s=s, hm=hm: st3_group_ws(s, hm) for hm in range(KH)]
                ems = []
                for hb in range(H // HB):
                    if hb > 0:
                        ems.append(lambda s=s, hb=hb: emit_w3(s, hb))
                    for t in range(len(st[s]["tts"])):
                        ems.append(lambda s=s, hb=hb, t=t: st3_group(s, hb, t))
                return ems

            def interleave(a_ems, b_ems):
                """Emit a and b emitter lists merged evenly (b spread among a)."""
                na, nb = len(a_ems), len(b_ems)
                bi = 0
                for i, a in enumerate(a_ems):
                    while bi < nb and bi * na <= i * nb:
                        b_ems[bi]()
                        bi += 1
                    a()
                while bi < nb:
                    b_ems[bi]()
                    bi += 1

            # ---- prime all three DMA rings with tiny transfers so the
            # ring-setup latency (1.6-4us) is paid during the boot barriers,
            # not in front of the first real weight loads ----
            for qi, eng in enumerate((nc.gpsimd, nc.sync, nc.scalar)):
                dt = p_b.tile([1, 64], MM_DT, tag=f"prime{qi}", name=f"prime{qi}")
                eng.dma_start(out=dt[:], in_=w1_d[0][0][0:1, 0:64])

            # ---- head of the gpsimd DMA queue: slot 0's first w1 tiles so
            # the PE can start the moment warmup ends ----
            for m in range(8):
                w1t = p_w1.tile([128, H], MM_DT, tag="w1", name=f"w1pre_{m}")
                nc.sync.dma_start(out=w1t[:], in_=w1_d[0][m])
                pre_w1[m] = w1t

            # ---- PE pre-warm: ~4us of junk matmuls so the HAM clock-gate
            # reaches 2.4 GHz while the first input DMAs land ----
            junk = p_b.tile([128, 512], MM_DT, tag="warm", name="warm_src")
            nc.vector.memset(junk[:], 0.0)
            psw = p_ps.tile([128, 512], F32, tag="ps", name="warm_ps")
            for i in range(10):
                nc.tensor.matmul(psw[:], junk[:, 0:128], junk[:],
                                 start=(i == 0), stop=(i == 9))

            # ---- emission schedule: st3(s-1) interleaves with st1(s) ----
            emit_loads(0)
            # prefetch slot-0 stage-2 weights on the scalar queue (idle at
            # startup) so st2(0) isn't gated by the sync queue's ramp
            for m in range(8):
                w2t = p_w2.tile([128, M], MM_DT, tag="w2", name=f"w2pre_{m}")
                nc.scalar.dma_start(out=w2t[:], in_=w2_d[0][m])
                pre_w2[m] = w2t
            prev_st3 = []
            for s in range(S):
                if s > 0:
                    emit_loads(s)
                interleave([lambda s=s, m=m: st1_group(s, m) for m in range(KM)],
                           prev_st3)
                if not prescaled:
                    emit_w3(s, 0)    # prefetch stage-3 hb=0 weights early
                for m in range(KM):
                    st2_group(s, m)
                prev_st3 = st3_emitters(s)
            for em in prev_st3:
                em()

    if legalize:
        _legalize_waits(nc)
    return nc


# ---------------------------------------------------------------------------
def _run_spmd(CS, prescaled, in_maps):
    """Compile + run on cores 0-7. On a transient device failure (e.g.
    NRT_EXEC_UNIT_UNRECOVERABLE from a stale runtime state), retry in a
    fresh subprocess whose NRT session starts clean."""
    try:
        nc = _build_program(CS, prescaled=prescaled)
        return run_bass_kernel_spmd(nc, in_maps, list(range(N_CORES))).results
    except Exception:
        import os
        import pickle
        import subprocess
        import sys
        import tempfile
        d = tempfile.mkdtemp()
        inp, outp = os.path.join(d, "in.pkl"), os.path.join(d, "out.pkl")
        with open(inp, "wb") as f:
            pickle.dump((CS, prescaled, in_maps), f)
        code = (
            "import pickle, sys\n"
            f"sys.path.insert(0, {os.path.dirname(os.path.abspath(__file__))!r})\n"
            "import kernel as K\n"
            f"CS, prescaled, in_maps = pickle.load(open({inp!r}, 'rb'))\n"
            "nc = K._build_program(CS, prescaled=prescaled)\n"
            "from concourse.bass_utils import run_bass_kernel_spmd\n"
            "r = run_bass_kernel_spmd(nc, in_maps, list(range(K.N_CORES))).results\n"
            f"pickle.dump(r, open({outp!r}, 'wb'))\n"
        )
        err = None
        for _ in range(2):
            try:
                subprocess.run([sys.executable, "-c", code], check=True,
                               timeout=1800)
                with open(outp, "rb") as f:
                    return pickle.load(f)
            except Exception as e:
                err = e
        raise err


def kernel(x, genre_embed, rms_w, wg_W, wg_b, gg_W, gg_b, W1, b1, W2, b2, W3, b3):
    x = np.asarray(x, np.float32)
    B, S_, _ = x.shape
    T = B * S_
    x2d = np.ascontiguousarray(x.reshape(T, H))
    W1 = np.asarray(W1, np.float32)
    W2 = np.asarray(W2, np.float32)
    W3 = np.asarray(W3, np.float32)

    if MM_DT == BF16:
        import ml_dtypes
        host_dt = ml_dtypes.bfloat16
    else:
        host_dt = np.float32

    cw = _route(x2d, np.asarray(genre_embed, np.float32), np.asarray(rms_w, np.float32),
                np.asarray(wg_W, np.float32), np.asarray(wg_b, np.float32),
                np.asarray(gg_W, np.float32), np.asarray(gg_b, np.float32), B, S_)
    E = cw.shape[1]
    tok_by_e = [np.nonzero(cw[:, e])[0] for e in range(E)]
    counts = [len(t) for t in tok_by_e]
    CS, slots = _partition(counts)

    # prescaled mode is exact when the in-MLP biases are zero (cw >= 0
    # commutes with relu); b3 is always applied on the host via cw @ b3
    prescaled = not (np.any(np.asarray(b1)) or np.any(np.asarray(b2)))

    # pre-tile weights once per expert (shared across cores)
    used = set(s[0] for core in slots for s in core if s)
    w1_tiled, w2_tiled, w3_tiled = {}, {}, {}
    for e in used:
        w1_tiled[e] = np.ascontiguousarray(
            W1[e].reshape(KH, 128, KM, 128).transpose(2, 1, 0, 3).reshape(KM, 128, H)).astype(host_dt)
        w2_tiled[e] = np.ascontiguousarray(
            W2[e].reshape(KM, 128, KM, 128).transpose(2, 1, 0, 3).reshape(KM, 128, M)).astype(host_dt)
        if prescaled:
            w3_tiled[e] = np.ascontiguousarray(
                W3[e].reshape(KM, 128, KH, 128).transpose(2, 1, 0, 3).reshape(KH, 128, M)).astype(host_dt)
        else:
            w3_tiled[e] = W3[e].astype(host_dt)

    e_any = next(iter(used))
    in_maps = []
    meta = []
    for core in range(N_CORES):
        im = {}
        cmeta = []
        for si in range(len(CS)):
            C = CS[si]
            slot = slots[core][si]
            e, lo, hi = slot if slot else (e_any, 0, 0)
            idx = tok_by_e[e][lo:hi]
            n = len(idx)
            xt = np.zeros((H, C), host_dt)
            if prescaled:
                xt[:, :n] = (x2d[idx] * cw[idx, e][:, None]).T.astype(host_dt)
            else:
                xt[:, :n] = x2d[idx].T.astype(host_dt)
            im[f"XT{si}"] = np.ascontiguousarray(
                xt.reshape(KH, 128, C).transpose(1, 0, 2).reshape(128, KH * C))
            im[f"W1{si}"] = w1_tiled[e]
            im[f"W2{si}"] = w2_tiled[e]
            im[f"W3{si}"] = w3_tiled[e]
            if not prescaled:
                cwc = np.zeros((C,), np.float32)
                cwc[:n] = cw[idx, e]
                im[f"B1{si}"] = np.asarray(b1[e], np.float32)
                im[f"B2{si}"] = np.asarray(b2[e], np.float32)
                im[f"CW{si}"] = cwc
            cmeta.append(idx)
        in_maps.append(im)
        meta.append(cmeta)

    results = _run_spmd(CS, prescaled, in_maps)

    out2d = cw @ np.asarray(b3, np.float32)      # bias-3 combine term [T, H]
    for core in range(N_CORES):
        for si, idx in enumerate(meta[core]):
            y = results[core][f"Y{si}"]
            if prescaled:
                out2d[idx] += np.asarray(y[:, :len(idx)], np.float32).T
            else:
                out2d[idx] += y[:len(idx)]
    return out2d.reshape(B, S_, H).astype(np.float32)



# revision 54
# speedup vs baseline: 1.0227x; 1.0227x over previous
"""MoE genre-gate kernel for 8 Trainium2 NeuronCores.

Strategy (expert-parallel with token dispatch, per sharding hint):
  - Routing (RMSNorm -> word+genre gate -> softmax -> top-2) is computed on
    host in float64: it is 0.03% of the FLOPs and produces the data-dependent
    dispatch tables (the stand-in for all-to-all).
  - The 8192 (token, expert) pairs are grouped per expert and packed into
    per-core slots by a capacity-profile search (_partition): it finds slot
    capacities CS (identical on every core = SPMD) minimizing total streamed
    columns, then bin-packs expert token chunks into the (core, slot) grid.
    For the seed-0 data this yields CS=[416, 384, 288] (1088 cols/core vs
    the 1024 ideal).
  - Each core runs a dense 3-stage MLP over its expert slots in bf16
    (full PE stream rate, FWL-fast weight loads, half the HBM traffic of
    fp32; ~3e-3 end-to-end rel err) with fp32 PSUM accumulation.  With zero
    in-MLP biases the host pre-scales each token row by its combine weight
    cw >= 0 (commutes with relu), so padding rows are exactly zero and
    stage 3 runs weight-stationary; a general biased fallback path is kept.
  - DMA: weight streams are spread over the three DMA-capable queues
    (w1+xt-tail on gpsimd, w2 on sync, w3+xt on scalar) with 8-deep tile
    pools; slot-0's first 8 w1 tiles (sync) and 8 w2 tiles (scalar) are
    prefetched at the program head so the PE never starves during the
    DMA-ring ramp-up.  10 junk matmuls warm the HAM clock-gate to 2.4 GHz
    while the first loads land, with no idle gap into the real stream.
  - Stage 3 of slot s-1 is software-interleaved with stage 1 of slot s to
    keep the PE busy across stage boundaries; stage-3 outputs drain via the
    idle DVE (vector) engine as bf16.
  - Host scatter-adds the per-pair outputs back to [B,S,H] and adds the
    (cw @ b3) bias term.

Measured: 339.9us (baseline) -> ~256us; PE streams at its 1-col/cycle
roofline (232us for 1088 cols x 512 matmul-cycles/col) inside the body.

Hardcoded problem shape: B=2, S=2048, H=1024, G=256, E=8, M=2048, top-2.
"""

import numpy as np

import concourse.bass as bass
import concourse.tile as tile
from concourse import mybir
from concourse.bass_utils import run_bass_kernel_spmd

TOP_K = 2
EPS = 1e-6
N_CORES = 8
H = 1024
M = 2048
KH, KM = H // 128, M // 128
F32R = mybir.dt.float32r
F32 = mybir.dt.float32
BF16 = mybir.dt.bfloat16
MM_DT = BF16          # matmul dtype (bf16: full PE rate, FWL 2x weight loads, half DMA)


# ---------------------------------------------------------------------------
# walrus in this container accepts only ONE sync-wait command per
# instruction; Tile emits up to ~10.  Split extras onto standalone NoOps on
# the same engine, inserted immediately before the instruction, which
# preserves per-engine program order and therefore semantics.
_ctr = [0]


def _legalize_waits(nc, max_waits=1):
    for f in nc.m.functions:
        for blk in f.blocks:
            out = []
            for inst in blk.instructions:
                si = inst.sync_info
                if si is not None and len(si.on_wait) > max_waits:
                    waits = list(si.on_wait)
                    extra, keep = waits[:-max_waits], waits[-max_waits:]
                    for w in extra:
                        _ctr[0] += 1
                        out.append(mybir.InstNoOp(
                            name=f"waitsplit-{_ctr[0]}",
                            engine=inst.engine, ins=[], outs=[],
                            sync_info=mybir.SyncInfo(on_wait=[w], on_update=[]),
                        ))
                    inst.sync_info = mybir.SyncInfo(
                        on_wait=keep, on_update=list(si.on_update))
                out.append(inst)
            blk.instructions = out


# ---------------------------------------------------------------------------
def _route(x2d, genre_embed, rms_w, wg_W, wg_b, gg_W, gg_b, B, S):
    """Host gating in float64. Returns combine weights [T, E] (zero outside
    top-2)."""
    xd = x2d.astype(np.float64)
    var = np.mean(xd * xd, axis=-1, keepdims=True)
    xn = rms_w.astype(np.float64) * (xd / np.sqrt(var + EPS))
    gate = xn @ wg_W.astype(np.float64) + wg_b.astype(np.float64)
    gg = genre_embed.astype(np.float64)[:, 0, :] @ gg_W.astype(np.float64) \
        + gg_b.astype(np.float64)                       # [B, E]
    gate = gate.reshape(B, S, -1) + gg[:, None, :]
    gate = gate.reshape(B * S, -1)
    gate -= gate.max(axis=-1, keepdims=True)
    p = np.exp(gate)
    p /= p.sum(axis=-1, keepdims=True)
    top2 = np.argsort(-p, axis=-1)[:, :TOP_K]
    cw = np.zeros_like(p)
    rows = np.arange(p.shape[0])[:, None]
    cw[rows, top2] = p[rows, top2]
    return cw.astype(np.float32)


def _fit(counts, CS):
    """Cover counts with bins (N_CORES bins of capacity CS[j] per class j);
    each bin holds one contiguous chunk of one expert.  Returns n[e][j] bin
    usage, or None if infeasible.  DFS over experts (desc count) with a
    global waste budget and failed-state memo."""
    E, K = len(counts), len(CS)
    slack = N_CORES * sum(CS) - sum(counts)
    if slack < 0:
        return None
    order = sorted(range(E), key=lambda e: -counts[e])
    failed = set()

    def combos(need, avail):
        """Minimal bin combos covering `need`, sorted by waste."""
        out = []

        def rec(j, left, cur):
            if left <= 0:
                out.append(tuple(cur) + (0,) * (K - len(cur)))
                return
            if j == K:
                return
            if sum(avail[i] * CS[i] for i in range(j, K)) < left:
                return
            for n in range(min(avail[j], -(-left // CS[j])), -1, -1):
                rec(j + 1, left - n * CS[j], cur + [n])

        rec(0, need, [])
        mins = [c for c in out
                if all(c[j] == 0 or sum(ci * CS[i] for i, ci in enumerate(c))
                       - CS[j] < need for j in range(K))]
        mins.sort(key=lambda c: sum(ci * CS[i] for i, ci in enumerate(c)))
        return mins

    def dfs(ei, avail, budget):
        if ei == E:
            return []
        key = (ei, tuple(avail))
        if key in failed:
            return None
        e = order[ei]
        for c in combos(counts[e], avail):
            w = sum(ci * CS[i] for i, ci in enumerate(c)) - counts[e]
            if w > budget:
                break
            rest = dfs(ei + 1, [a - ci for a, ci in zip(avail, c)],
                       budget - w)
            if rest is not None:
                return [c] + rest
        failed.add(key)
        return None

    sol = dfs(0, [N_CORES] * K, slack)
    if sol is None:
        return None
    n = [[0] * K for _ in range(E)]
    for ei, c in enumerate(sol):
        n[order[ei]] = list(c)
    return n


def _partition(counts):
    """Choose per-core slot capacities CS and assign expert token chunks to
    (core, slot) bins, minimizing total streamed columns (sum CS) with a
    small penalty per extra slot class (extra weight DMA + ldweights).

    Returns (CS, slots): slots[core][j] = (expert, lo, hi) or None."""
    E = len(counts)
    cand = []
    caps = list(range(512, 63, -32))
    import itertools
    for K in (2, 3, 4):
        for prof in itertools.combinations_with_replacement(caps, K):
            prof = tuple(sorted(prof, reverse=True))
            score = (sum(prof) + 64 * (K - 2), K, -min(prof))
            cand.append((score, prof))
    cand.sort()
    seen = set()
    best = None
    for score, prof in cand:
        if prof in seen:
            continue
        seen.add(prof)
        if best is not None and score[0] >= best[0][0]:
            break
        n = _fit(counts, list(prof))
        if n is not None:
            best = (score, prof, n)
            break
    assert best is not None, "no feasible capacity profile"
    _, CS, n = best
    K = len(CS)
    # materialize chunks: fill each expert's bins largest-class-first
    slots = [[None] * K for _ in range(N_CORES)]
    next_core = [0] * K
    for e in range(E):
        lo = 0
        for j in range(K):
            for _ in range(n[e][j]):
                hi = min(lo + CS[j], counts[e])
                slots[next_core[j]][j] = (e, lo, hi)
                next_core[j] += 1
                lo = hi
        assert lo == counts[e]
    return list(CS), slots


def _token_chunks(C):
    """Split C into matmul moving-dim chunks, each <=512 (PSUM bank) and as
    equal as possible (>=256 keeps float32r at full rate)."""
    assert C % 2 == 0
    n = -(-C // 512)
    h = C // 2
    base, rem = divmod(h, n)
    sizes = [2 * (base + (1 if i < rem else 0)) for i in range(n)]
    assert sum(sizes) == C and all(s <= 512 and s % 2 == 0 for s in sizes)
    return sizes


# ---------------------------------------------------------------------------
def _build_program(CS, prescaled=False, legalize=True):
    """Emit the SPMD Bass program; CS = per-slot-class capacities.

    prescaled=True (valid when b1==b2==0): host pre-scales x rows by cw
    (cw>=0 commutes with relu), so no bias/cw tiles are needed and stage 3
    runs weight-stationary with output layout [H, C] (y transposed)."""
    S = len(CS)
    nc = bass.Bass()
    xt_d = [nc.dram_tensor(f"XT{s}", [128, KH * CS[s]], MM_DT, kind="ExternalInput") for s in range(S)]
    w1_d = [nc.dram_tensor(f"W1{s}", [KM, 128, H], MM_DT, kind="ExternalInput") for s in range(S)]
    w2_d = [nc.dram_tensor(f"W2{s}", [KM, 128, M], MM_DT, kind="ExternalInput") for s in range(S)]
    if prescaled:
        w3_d = [nc.dram_tensor(f"W3{s}", [KH, 128, M], MM_DT, kind="ExternalInput") for s in range(S)]
        y_d = [nc.dram_tensor(f"Y{s}", [H, CS[s]], BF16, kind="ExternalOutput") for s in range(S)]
    else:
        w3_d = [nc.dram_tensor(f"W3{s}", [M, H], MM_DT, kind="ExternalInput") for s in range(S)]
        y_d = [nc.dram_tensor(f"Y{s}", [CS[s], H], F32, kind="ExternalOutput") for s in range(S)]
        b1_d = [nc.dram_tensor(f"B1{s}", [M], F32, kind="ExternalInput") for s in range(S)]
        b2_d = [nc.dram_tensor(f"B2{s}", [M], F32, kind="ExternalInput") for s in range(S)]
        cw_d = [nc.dram_tensor(f"CW{s}", [CS[s]], F32, kind="ExternalInput") for s in range(S)]

    HB = 512 if len(CS) >= 3 else 256

    with tile.TileContext(nc) as tc:
        with (
            tc.tile_pool(name="xt", bufs=2) as p_xt,
            tc.tile_pool(name="w", bufs=8) as p_w,
            tc.tile_pool(name="h", bufs=1) as p_h,
            tc.tile_pool(name="bias", bufs=1) as p_b,
            tc.tile_pool(name="y", bufs=4) as p_y,
            tc.tile_pool(name="ps", bufs=8, space="PSUM") as p_ps,
        ):
            p_w1 = p_w2 = p_w3 = p_w
            p_h1 = p_h2 = p_h
            st = [dict() for _ in range(S)]   # per-slot tiles/geometry

            def emit_loads(s):
                C = CS[s]
                v = st[s]
                v["tcs"] = _token_chunks(C)
                v["tco"] = np.cumsum([0] + v["tcs"]).tolist()
                v["tts"] = [(i * 128, min(128, C - i * 128)) for i in range(-(-C // 128))]
                xt_all = p_xt.tile([128, KH * C], MM_DT, tag="xt", name=f"xt_{s}")
                half = KH * C // 2
                nc.scalar.dma_start(out=xt_all[:, :half], in_=xt_d[s][:, :half])
                nc.scalar.dma_start(out=xt_all[:, half:], in_=xt_d[s][:, half:])
                v["xt_all"] = xt_all
                if prescaled:
                    v["h1"] = p_h1.tile([128, KM * C], MM_DT, tag="h1", name=f"h1_{s}")
                    v["h2"] = p_h2.tile([128, KM * C], MM_DT, tag="h2", name=f"h2_{s}")
                    return
                v["b1t"] = [p_b.tile([128, 1], F32, tag=f"b1_{s}_{m}", name=f"b1t_{s}_{m}") for m in range(KM)]
                v["b2t"] = [p_b.tile([128, 1], F32, tag=f"b2_{s}_{m}", name=f"b2t_{s}_{m}") for m in range(KM)]
                for m in range(KM):
                    nc.scalar.dma_start(out=v["b1t"][m][:], in_=b1_d[s][m * 128:(m + 1) * 128].rearrange("(p one) -> p one", one=1))
                    nc.scalar.dma_start(out=v["b2t"][m][:], in_=b2_d[s][m * 128:(m + 1) * 128].rearrange("(p one) -> p one", one=1))
                v["cwt"] = []
                for t, (t0, tn) in enumerate(v["tts"]):
                    v["cwt"].append(p_b.tile([tn, 1], F32, tag=f"cw_{s}_{t}", name=f"cwt_{s}_{t}"))
                    nc.scalar.dma_start(out=v["cwt"][t][:], in_=cw_d[s][t0:t0 + tn].rearrange("(p one) -> p one", one=1))
                v["h1"] = [p_h1.tile([128, C], MM_DT, tag=f"h1_{m}", name=f"h1_{s}_{m}") for m in range(KM)]
                v["h2"] = [p_h2.tile([128, C], MM_DT, tag=f"h2_{m}", name=f"h2_{s}_{m}") for m in range(KM)]

            pre_w1 = {}

            def h1s(v, s, m, a, b):
                if prescaled:
                    return v["h1"][:, m * CS[s] + a:m * CS[s] + b]
                return v["h1"][m][:, a:b]

            def h2s(v, s, m, a, b):
                if prescaled:
                    return v["h2"][:, m * CS[s] + a:m * CS[s] + b]
                return v["h2"][m][:, a:b]

            def st1_group(s, m):
                v = st[s]
                if s == 0 and m in pre_w1:
                    w1t = pre_w1[m]
                else:
                    w1t = p_w1.tile([128, H], MM_DT, tag="w1", name=f"w1t_{s}_{m}")
                    nc.gpsimd.dma_start(out=w1t[:], in_=w1_d[s][m])
                pss = [p_ps.tile([128, tcz], F32, tag="ps", name=f"ps1_{s}_{m}_{ci}")
                       for ci, tcz in enumerate(v["tcs"])]
                C = CS[s]
                for k in range(KH):
                    for ci in range(len(v["tcs"])):
                        nc.tensor.matmul(
                            pss[ci][:], w1t[:, k * 128:(k + 1) * 128],
                            v["xt_all"][:, k * C + v["tco"][ci]:k * C + v["tco"][ci + 1]],
                            start=(k == 0), stop=(k == KH - 1))
                for ci in range(len(v["tcs"])):
                    nc.scalar.activation(
                        h1s(v, s, m, v["tco"][ci], v["tco"][ci + 1]), pss[ci][:],
                        mybir.ActivationFunctionType.Relu,
                        **({} if prescaled else {"bias": v["b1t"][m][:, 0:1]}))

            pre_w2 = {}

            def st2_group(s, m):
                v = st[s]
                if s == 0 and m in pre_w2:
                    w2t = pre_w2[m]
                else:
                    w2t = p_w2.tile([128, M], MM_DT, tag="w2", name=f"w2t_{s}_{m}")
                    nc.sync.dma_start(out=w2t[:], in_=w2_d[s][m])
                pss = [p_ps.tile([128, tcz], F32, tag="ps", name=f"ps2_{s}_{m}_{ci}")
                       for ci, tcz in enumerate(v["tcs"])]
                for k in range(KM):
                    for ci in range(len(v["tcs"])):
                        nc.tensor.matmul(
                            pss[ci][:], w2t[:, k * 128:(k + 1) * 128],
                            h1s(v, s, k, v["tco"][ci], v["tco"][ci + 1]),
                            start=(k == 0), stop=(k == KM - 1))
                for ci in range(len(v["tcs"])):
                    nc.scalar.activation(
                        h2s(v, s, m, v["tco"][ci], v["tco"][ci + 1]), pss[ci][:],
                        mybir.ActivationFunctionType.Relu,
                        **({} if prescaled else {"bias": v["b2t"][m][:, 0:1]}))

            def emit_w3(s, hb):
                w3t = [p_w3.tile([128, HB], MM_DT, tag=f"w3_{k % 4}", name=f"w3t_{s}_{hb}_{k}") for k in range(KM)]
                for k in range(KM):
                    nc.gpsimd.dma_start(
                        out=w3t[k][:],
                        in_=w3_d[s][k * 128:(k + 1) * 128, hb * HB:(hb + 1) * HB])
                st[s][f"w3_{hb}"] = w3t

            def st3_group(s, hb, t):
                v = st[s]
                t0, tn = v["tts"][t]
                w3t = v[f"w3_{hb}"]
                ps = p_ps.tile([tn, HB], F32, tag="ps", name=f"ps3_{s}_{hb}_{t}")
                for k in range(KM):
                    nc.tensor.matmul(
                        ps[:], h2s(v, s, k, t0, t0 + tn), w3t[k][:],
                        start=(k == 0), stop=(k == KM - 1))
                yt = p_y.tile([tn, HB], F32, tag="y", name=f"yt_{s}_{hb}_{t}")
                nc.scalar.activation(
                    yt[:], ps[:], mybir.ActivationFunctionType.Copy,
                    scale=v["cwt"][t][:, 0:1])
                nc.scalar.dma_start(
                    out=y_d[s][t0:t0 + tn, hb * HB:(hb + 1) * HB],
                    in_=yt[:])

            def st3_group_ws(s, hm):
                """Weight-stationary stage 3 (prescaled mode): out y_T[h, tok]."""
                v = st[s]
                w3t = p_w3.tile([128, M], MM_DT, tag="w3ws", name=f"w3ws_{s}_{hm}")
                nc.scalar.dma_start(out=w3t[:], in_=w3_d[s][hm])
                pss = [p_ps.tile([128, tcz], F32, tag="ps", name=f"ps3_{s}_{hm}_{ci}")
                       for ci, tcz in enumerate(v["tcs"])]
                for k in range(KM):
                    for ci in range(len(v["tcs"])):
                        nc.tensor.matmul(
                            pss[ci][:], w3t[:, k * 128:(k + 1) * 128],
                            h2s(v, s, k, v["tco"][ci], v["tco"][ci + 1]),
                            start=(k == 0), stop=(k == KM - 1))
                for ci, tcz in enumerate(v["tcs"]):
                    yt = p_y.tile([128, tcz], BF16, tag="y", name=f"yt_{s}_{hm}_{ci}")
                    nc.vector.tensor_scalar_mul(yt[:], pss[ci][:], 1.0)
                    nc.scalar.dma_start(
                        out=y_d[s][hm * 128:(hm + 1) * 128, v["tco"][ci]:v["tco"][ci + 1]],
                        in_=yt[:])

            def st3_emitters(s):
                if prescaled:
                    return [lambda s=s, hm=hm: st3_group_ws(s, hm) for hm in range(KH)]
                ems = []
                for hb in range(H // HB):
                    if hb > 0:
                        ems.append(lambda s=s, hb=hb: emit_w3(s, hb))
                    for t in range(len(st[s]["tts"])):
                        ems.append(lambda s=s, hb=hb, t=t: st3_group(s, hb, t))
                return ems

            def interleave(a_ems, b_ems):
                """Emit a and b emitter lists merged evenly (b spread among a)."""
                na, nb = len(a_ems), len(b_ems)
                bi = 0
                for i, a in enumerate(a_ems):
                    while bi < nb and bi * na <= i * nb:
                        b_ems[bi]()
                        bi += 1
                    a()
                while bi < nb:
                    b_ems[bi]()
                    bi += 1

            # ---- prime all three DMA rings with tiny transfers so the
            # ring-setup latency (1.6-4us) is paid during the boot barriers,
            # not in front of the first real weight loads ----
            for qi, eng in enumerate((nc.gpsimd, nc.sync, nc.scalar)):
                dt = p_b.tile([1, 64], MM_DT, tag=f"prime{qi}", name=f"prime{qi}")
                eng.dma_start(out=dt[:], in_=w1_d[0][0][0:1, 0:64])

            # ---- head of the gpsimd DMA queue: slot 0's first w1 tiles so
            # the PE can start the moment warmup ends ----
            for m in range(8):
                w1t = p_w1.tile([128, H], MM_DT, tag="w1", name=f"w1pre_{m}")
                nc.sync.dma_start(out=w1t[:], in_=w1_d[0][m])
                pre_w1[m] = w1t

            # ---- PE pre-warm: ~4us of junk matmuls so the HAM clock-gate
            # reaches 2.4 GHz while the first input DMAs land ----
            junk = p_b.tile([128, 512], MM_DT, tag="warm", name="warm_src")
            nc.vector.memset(junk[:], 0.0)
            psw = p_ps.tile([128, 512], F32, tag="ps", name="warm_ps")
            for i in range(10):
                nc.tensor.matmul(psw[:], junk[:, 0:128], junk[:],
                                 start=(i == 0), stop=(i == 9))

            # ---- emission schedule: st3(s-1) interleaves with st1(s) ----
            emit_loads(0)
            # prefetch slot-0 stage-2 weights on the scalar queue (idle at
            # startup) so st2(0) isn't gated by the sync queue's ramp
            for m in range(8):
                w2t = p_w2.tile([128, M], MM_DT, tag="w2", name=f"w2pre_{m}")
                nc.scalar.dma_start(out=w2t[:], in_=w2_d[0][m])
                pre_w2[m] = w2t
            prev_st3 = []
            for s in range(S):
                if s > 0:
                    emit_loads(s)
                interleave([lambda s=s, m=m: st1_group(s, m) for m in range(KM)],
                           prev_st3)
                if not prescaled:
                    emit_w3(s, 0)    # prefetch stage-3 hb=0 weights early
                for m in range(KM):
                    st2_group(s, m)
                prev_st3 = st3_emitters(s)
            for em in prev_st3:
                em()

    if legalize:
        _legalize_waits(nc)
    return nc


# ---------------------------------------------------------------------------
def _run_spmd(CS, prescaled, in_maps):
    """Compile + run on cores 0-7. On a transient device failure (e.g.
    NRT_EXEC_UNIT_UNRECOVERABLE from a stale runtime state), retry in a
    fresh subprocess whose NRT session starts clean."""
    try:
        nc = _build_program(CS, prescaled=prescaled)
        return run_bass_kernel_spmd(nc, in_maps, list(range(N_CORES))).results
    except Exception:
        import os
        import pickle
        import subprocess
        import sys
        import tempfile
        d = tempfile.mkdtemp()
        inp, outp = os.path.join(d, "in.pkl"), os.path.join(d, "out.pkl")
        with open(inp, "wb") as f:
            pickle.dump((CS, prescaled, in_maps), f)
        code = (
            "import pickle, sys\n"
            f"sys.path.insert(0, {os.path.dirname(os.path.abspath(__file__))!r})\n"
            "import kernel as K\n"
            f"CS, prescaled, in_maps = pickle.load(open({inp!r}, 'rb'))\n"
            "nc = K._build_program(CS, prescaled=prescaled)\n"
            "from concourse.bass_utils import run_bass_kernel_spmd\n"
            "r = run_bass_kernel_spmd(nc, in_maps, list(range(K.N_CORES))).results\n"
            f"pickle.dump(r, open({outp!r}, 'wb'))\n"
        )
        err = None
        for _ in range(2):
            try:
                subprocess.run([sys.executable, "-c", code], check=True,
                               timeout=1800)
                with open(outp, "rb") as f:
                    return pickle.load(f)
            except Exception as e:
                err = e
        raise err


def kernel(x, genre_embed, rms_w, wg_W, wg_b, gg_W, gg_b, W1, b1, W2, b2, W3, b3):
    x = np.asarray(x, np.float32)
    B, S_, _ = x.shape
    T = B * S_
    x2d = np.ascontiguousarray(x.reshape(T, H))
    W1 = np.asarray(W1, np.float32)
    W2 = np.asarray(W2, np.float32)
    W3 = np.asarray(W3, np.float32)

    if MM_DT == BF16:
        import ml_dtypes
        host_dt = ml_dtypes.bfloat16
    else:
        host_dt = np.float32

    cw = _route(x2d, np.asarray(genre_embed, np.float32), np.asarray(rms_w, np.float32),
                np.asarray(wg_W, np.float32), np.asarray(wg_b, np.float32),
                np.asarray(gg_W, np.float32), np.asarray(gg_b, np.float32), B, S_)
    E = cw.shape[1]
    tok_by_e = [np.nonzero(cw[:, e])[0] for e in range(E)]
    counts = [len(t) for t in tok_by_e]
    CS, slots = _partition(counts)

    # prescaled mode is exact when the in-MLP biases are zero (cw >= 0
    # commutes with relu); b3 is always applied on the host via cw @ b3
    prescaled = not (np.any(np.asarray(b1)) or np.any(np.asarray(b2)))

    # pre-tile weights once per expert (shared across cores)
    used = set(s[0] for core in slots for s in core if s)
    w1_tiled, w2_tiled, w3_tiled = {}, {}, {}
    for e in used:
        w1_tiled[e] = np.ascontiguousarray(
            W1[e].reshape(KH, 128, KM, 128).transpose(2, 1, 0, 3).reshape(KM, 128, H)).astype(host_dt)
        w2_tiled[e] = np.ascontiguousarray(
            W2[e].reshape(KM, 128, KM, 128).transpose(2, 1, 0, 3).reshape(KM, 128, M)).astype(host_dt)
        if prescaled:
            w3_tiled[e] = np.ascontiguousarray(
                W3[e].reshape(KM, 128, KH, 128).transpose(2, 1, 0, 3).reshape(KH, 128, M)).astype(host_dt)
        else:
            w3_tiled[e] = W3[e].astype(host_dt)

    e_any = next(iter(used))
    in_maps = []
    meta = []
    for core in range(N_CORES):
        im = {}
        cmeta = []
        for si in range(len(CS)):
            C = CS[si]
            slot = slots[core][si]
            e, lo, hi = slot if slot else (e_any, 0, 0)
            idx = tok_by_e[e][lo:hi]
            n = len(idx)
            xt = np.zeros((H, C), host_dt)
            if prescaled:
                xt[:, :n] = (x2d[idx] * cw[idx, e][:, None]).T.astype(host_dt)
            else:
                xt[:, :n] = x2d[idx].T.astype(host_dt)
            im[f"XT{si}"] = np.ascontiguousarray(
                xt.reshape(KH, 128, C).transpose(1, 0, 2).reshape(128, KH * C))
            im[f"W1{si}"] = w1_tiled[e]
            im[f"W2{si}"] = w2_tiled[e]
            im[f"W3{si}"] = w3_tiled[e]
            if not prescaled:
                cwc = np.zeros((C,), np.float32)
                cwc[:n] = cw[idx, e]
                im[f"B1{si}"] = np.asarray(b1[e], np.float32)
                im[f"B2{si}"] = np.asarray(b2[e], np.float32)
                im[f"CW{si}"] = cwc
            cmeta.append(idx)
        in_maps.append(im)
        meta.append(cmeta)

    results = _run_spmd(CS, prescaled, in_maps)

    out2d = cw @ np.asarray(b3, np.float32)      # bias-3 combine term [T, H]
    for core in range(N_CORES):
        for si, idx in enumerate(meta[core]):
            y = results[core][f"Y{si}"]
            if prescaled:
                out2d[idx] += np.asarray(y[:, :len(idx)], np.float32).T
            else:
                out2d[idx] += y[:len(idx)]
    return out2d.reshape(B, S_, H).astype(np.float32)



# revision 55
# speedup vs baseline: 1.0445x; 1.0213x over previous
"""MoE genre-gate kernel for 8 Trainium2 NeuronCores.

Strategy (expert-parallel with token dispatch, per sharding hint):
  - Routing (RMSNorm -> word+genre gate -> softmax -> top-2) is computed on
    host in float64: it is 0.03% of the FLOPs and produces the data-dependent
    dispatch tables (the stand-in for all-to-all).
  - The 8192 (token, expert) pairs are grouped per expert and packed into
    per-core slots by a capacity-profile search (_partition): it finds slot
    capacities CS (identical on every core = SPMD) minimizing total streamed
    columns, then bin-packs expert token chunks into the (core, slot) grid.
    For the seed-0 data this yields CS=[416, 384, 288] (1088 cols/core vs
    the 1024 ideal).
  - Each core runs a dense 3-stage MLP over its expert slots in bf16
    (full PE stream rate, FWL-fast weight loads, half the HBM traffic of
    fp32; ~3e-3 end-to-end rel err) with fp32 PSUM accumulation.  With zero
    in-MLP biases the host pre-scales each token row by its combine weight
    cw >= 0 (commutes with relu), so padding rows are exactly zero and
    stage 3 runs weight-stationary; a general biased fallback path is kept.
  - DMA: weight streams are spread over the three DMA-capable queues
    (w1+xt-tail on gpsimd, w2 on sync, w3+xt on scalar) with 8-deep tile
    pools; slot-0's first 8 w1 tiles (sync) and 8 w2 tiles (scalar) are
    prefetched at the program head so the PE never starves during the
    DMA-ring ramp-up.  10 junk matmuls warm the HAM clock-gate to 2.4 GHz
    while the first loads land, with no idle gap into the real stream.
  - Stage 3 of slot s-1 is software-interleaved with stage 1 of slot s to
    keep the PE busy across stage boundaries; stage-3 outputs drain via the
    idle DVE (vector) engine as bf16.
  - Host scatter-adds the per-pair outputs back to [B,S,H] and adds the
    (cw @ b3) bias term.

Measured: 339.9us (baseline) -> ~256us; PE streams at its 1-col/cycle
roofline (232us for 1088 cols x 512 matmul-cycles/col) inside the body.

Hardcoded problem shape: B=2, S=2048, H=1024, G=256, E=8, M=2048, top-2.
"""

import numpy as np

import concourse.bass as bass
import concourse.tile as tile
from concourse import mybir
from concourse.bass_utils import run_bass_kernel_spmd

TOP_K = 2
EPS = 1e-6
N_CORES = 8
H = 1024
M = 2048
KH, KM = H // 128, M // 128
F32R = mybir.dt.float32r
F32 = mybir.dt.float32
BF16 = mybir.dt.bfloat16
MM_DT = BF16          # matmul dtype (bf16: full PE rate, FWL 2x weight loads, half DMA)


# ---------------------------------------------------------------------------
# walrus in this container accepts only ONE sync-wait command per
# instruction; Tile emits up to ~10.  Split extras onto standalone NoOps on
# the same engine, inserted immediately before the instruction, which
# preserves per-engine program order and therefore semantics.
_ctr = [0]


def _legalize_waits(nc, max_waits=1):
    for f in nc.m.functions:
        for blk in f.blocks:
            out = []
            for inst in blk.instructions:
                si = inst.sync_info
                if si is not None and len(si.on_wait) > max_waits:
                    waits = list(si.on_wait)
                    extra, keep = waits[:-max_waits], waits[-max_waits:]
                    for w in extra:
                        _ctr[0] += 1
                        out.append(mybir.InstNoOp(
                            name=f"waitsplit-{_ctr[0]}",
                            engine=inst.engine, ins=[], outs=[],
                            sync_info=mybir.SyncInfo(on_wait=[w], on_update=[]),
                        ))
                    inst.sync_info = mybir.SyncInfo(
                        on_wait=keep, on_update=list(si.on_update))
                out.append(inst)
            blk.instructions = out


# ---------------------------------------------------------------------------
def _route(x2d, genre_embed, rms_w, wg_W, wg_b, gg_W, gg_b, B, S):
    """Host gating in float64. Returns combine weights [T, E] (zero outside
    top-2)."""
    xd = x2d.astype(np.float64)
    var = np.mean(xd * xd, axis=-1, keepdims=True)
    xn = rms_w.astype(np.float64) * (xd / np.sqrt(var + EPS))
    gate = xn @ wg_W.astype(np.float64) + wg_b.astype(np.float64)
    gg = genre_embed.astype(np.float64)[:, 0, :] @ gg_W.astype(np.float64) \
        + gg_b.astype(np.float64)                       # [B, E]
    gate = gate.reshape(B, S, -1) + gg[:, None, :]
    gate = gate.reshape(B * S, -1)
    gate -= gate.max(axis=-1, keepdims=True)
    p = np.exp(gate)
    p /= p.sum(axis=-1, keepdims=True)
    top2 = np.argsort(-p, axis=-1)[:, :TOP_K]
    cw = np.zeros_like(p)
    rows = np.arange(p.shape[0])[:, None]
    cw[rows, top2] = p[rows, top2]
    return cw.astype(np.float32)


def _fit(counts, CS):
    """Cover counts with bins (N_CORES bins of capacity CS[j] per class j);
    each bin holds one contiguous chunk of one expert.  Returns n[e][j] bin
    usage, or None if infeasible.  DFS over experts (desc count) with a
    global waste budget and failed-state memo."""
    E, K = len(counts), len(CS)
    slack = N_CORES * sum(CS) - sum(counts)
    if slack < 0:
        return None
    order = sorted(range(E), key=lambda e: -counts[e])
    failed = set()

    def combos(need, avail):
        """Minimal bin combos covering `need`, sorted by waste."""
        out = []

        def rec(j, left, cur):
            if left <= 0:
                out.append(tuple(cur) + (0,) * (K - len(cur)))
                return
            if j == K:
                return
            if sum(avail[i] * CS[i] for i in range(j, K)) < left:
                return
            for n in range(min(avail[j], -(-left // CS[j])), -1, -1):
                rec(j + 1, left - n * CS[j], cur + [n])

        rec(0, need, [])
        mins = [c for c in out
                if all(c[j] == 0 or sum(ci * CS[i] for i, ci in enumerate(c))
                       - CS[j] < need for j in range(K))]
        mins.sort(key=lambda c: sum(ci * CS[i] for i, ci in enumerate(c)))
        return mins

    def dfs(ei, avail, budget):
        if ei == E:
            return []
        key = (ei, tuple(avail))
        if key in failed:
            return None
        e = order[ei]
        for c in combos(counts[e], avail):
            w = sum(ci * CS[i] for i, ci in enumerate(c)) - counts[e]
            if w > budget:
                break
            rest = dfs(ei + 1, [a - ci for a, ci in zip(avail, c)],
                       budget - w)
            if rest is not None:
                return [c] + rest
        failed.add(key)
        return None

    sol = dfs(0, [N_CORES] * K, slack)
    if sol is None:
        return None
    n = [[0] * K for _ in range(E)]
    for ei, c in enumerate(sol):
        n[order[ei]] = list(c)
    return n


def _partition(counts):
    """Choose per-core slot capacities CS and assign expert token chunks to
    (core, slot) bins, minimizing total streamed columns (sum CS) with a
    small penalty per extra slot class (extra weight DMA + ldweights).

    Returns (CS, slots): slots[core][j] = (expert, lo, hi) or None."""
    E = len(counts)
    cand = []
    import itertools
    # caps below ~224 are matmul-overhead-heavy (N-cycle streaming no longer
    # dominates the fixed issue+ldweights cost), so floor the class size
    for K, caps in ((2, range(512, 223, -8)), (3, range(512, 223, -8)),
                    (4, range(512, 223, -32))):
        for prof in itertools.combinations_with_replacement(list(caps), K):
            prof = tuple(sorted(prof, reverse=True))
            score = (sum(prof) + 64 * (K - 2), K, -min(prof))
            cand.append((score, prof))
    cand.sort()
    seen = set()
    best = None
    for score, prof in cand:
        if prof in seen:
            continue
        seen.add(prof)
        if best is not None and score[0] >= best[0][0]:
            break
        n = _fit(counts, list(prof))
        if n is not None:
            best = (score, prof, n)
            break
    assert best is not None, "no feasible capacity profile"
    _, CS, n = best
    K = len(CS)
    # materialize chunks: fill each expert's bins largest-class-first
    slots = [[None] * K for _ in range(N_CORES)]
    next_core = [0] * K
    for e in range(E):
        lo = 0
        for j in range(K):
            for _ in range(n[e][j]):
                hi = min(lo + CS[j], counts[e])
                slots[next_core[j]][j] = (e, lo, hi)
                next_core[j] += 1
                lo = hi
        assert lo == counts[e]
    return list(CS), slots


def _token_chunks(C):
    """Split C into matmul moving-dim chunks, each <=512 (PSUM bank) and as
    equal as possible (>=256 keeps float32r at full rate)."""
    assert C % 2 == 0
    n = -(-C // 512)
    h = C // 2
    base, rem = divmod(h, n)
    sizes = [2 * (base + (1 if i < rem else 0)) for i in range(n)]
    assert sum(sizes) == C and all(s <= 512 and s % 2 == 0 for s in sizes)
    return sizes


# ---------------------------------------------------------------------------
def _build_program(CS, prescaled=False, legalize=True):
    """Emit the SPMD Bass program; CS = per-slot-class capacities.

    prescaled=True (valid when b1==b2==0): host pre-scales x rows by cw
    (cw>=0 commutes with relu), so no bias/cw tiles are needed and stage 3
    runs weight-stationary with output layout [H, C] (y transposed)."""
    S = len(CS)
    nc = bass.Bass()
    xt_d = [nc.dram_tensor(f"XT{s}", [128, KH * CS[s]], MM_DT, kind="ExternalInput") for s in range(S)]
    w1_d = [nc.dram_tensor(f"W1{s}", [KM, 128, H], MM_DT, kind="ExternalInput") for s in range(S)]
    w2_d = [nc.dram_tensor(f"W2{s}", [KM, 128, M], MM_DT, kind="ExternalInput") for s in range(S)]
    if prescaled:
        w3_d = [nc.dram_tensor(f"W3{s}", [KH, 128, M], MM_DT, kind="ExternalInput") for s in range(S)]
        y_d = [nc.dram_tensor(f"Y{s}", [H, CS[s]], BF16, kind="ExternalOutput") for s in range(S)]
    else:
        w3_d = [nc.dram_tensor(f"W3{s}", [M, H], MM_DT, kind="ExternalInput") for s in range(S)]
        y_d = [nc.dram_tensor(f"Y{s}", [CS[s], H], F32, kind="ExternalOutput") for s in range(S)]
        b1_d = [nc.dram_tensor(f"B1{s}", [M], F32, kind="ExternalInput") for s in range(S)]
        b2_d = [nc.dram_tensor(f"B2{s}", [M], F32, kind="ExternalInput") for s in range(S)]
        cw_d = [nc.dram_tensor(f"CW{s}", [CS[s]], F32, kind="ExternalInput") for s in range(S)]

    HB = 512 if len(CS) >= 3 else 256

    with tile.TileContext(nc) as tc:
        with (
            tc.tile_pool(name="xt", bufs=2) as p_xt,
            tc.tile_pool(name="w", bufs=8) as p_w,
            tc.tile_pool(name="h", bufs=1) as p_h,
            tc.tile_pool(name="bias", bufs=1) as p_b,
            tc.tile_pool(name="y", bufs=4) as p_y,
            tc.tile_pool(name="ps", bufs=8, space="PSUM") as p_ps,
        ):
            p_w1 = p_w2 = p_w3 = p_w
            p_h1 = p_h2 = p_h
            st = [dict() for _ in range(S)]   # per-slot tiles/geometry

            def emit_loads(s):
                C = CS[s]
                v = st[s]
                v["tcs"] = _token_chunks(C)
                v["tco"] = np.cumsum([0] + v["tcs"]).tolist()
                v["tts"] = [(i * 128, min(128, C - i * 128)) for i in range(-(-C // 128))]
                xt_all = p_xt.tile([128, KH * C], MM_DT, tag="xt", name=f"xt_{s}")
                half = KH * C // 2
                nc.scalar.dma_start(out=xt_all[:, :half], in_=xt_d[s][:, :half])
                nc.scalar.dma_start(out=xt_all[:, half:], in_=xt_d[s][:, half:])
                v["xt_all"] = xt_all
                if prescaled:
                    v["h1"] = p_h1.tile([128, KM * C], MM_DT, tag="h1", name=f"h1_{s}")
                    v["h2"] = p_h2.tile([128, KM * C], MM_DT, tag="h2", name=f"h2_{s}")
                    return
                v["b1t"] = [p_b.tile([128, 1], F32, tag=f"b1_{s}_{m}", name=f"b1t_{s}_{m}") for m in range(KM)]
                v["b2t"] = [p_b.tile([128, 1], F32, tag=f"b2_{s}_{m}", name=f"b2t_{s}_{m}") for m in range(KM)]
                for m in range(KM):
                    nc.scalar.dma_start(out=v["b1t"][m][:], in_=b1_d[s][m * 128:(m + 1) * 128].rearrange("(p one) -> p one", one=1))
                    nc.scalar.dma_start(out=v["b2t"][m][:], in_=b2_d[s][m * 128:(m + 1) * 128].rearrange("(p one) -> p one", one=1))
                v["cwt"] = []
                for t, (t0, tn) in enumerate(v["tts"]):
                    v["cwt"].append(p_b.tile([tn, 1], F32, tag=f"cw_{s}_{t}", name=f"cwt_{s}_{t}"))
                    nc.scalar.dma_start(out=v["cwt"][t][:], in_=cw_d[s][t0:t0 + tn].rearrange("(p one) -> p one", one=1))
                v["h1"] = [p_h1.tile([128, C], MM_DT, tag=f"h1_{m}", name=f"h1_{s}_{m}") for m in range(KM)]
                v["h2"] = [p_h2.tile([128, C], MM_DT, tag=f"h2_{m}", name=f"h2_{s}_{m}") for m in range(KM)]

            pre_w1 = {}

            def h1s(v, s, m, a, b):
                if prescaled:
                    return v["h1"][:, m * CS[s] + a:m * CS[s] + b]
                return v["h1"][m][:, a:b]

            def h2s(v, s, m, a, b):
                if prescaled:
                    return v["h2"][:, m * CS[s] + a:m * CS[s] + b]
                return v["h2"][m][:, a:b]

            def st1_group(s, m):
                v = st[s]
                if s == 0 and m in pre_w1:
                    w1t = pre_w1[m]
                else:
                    w1t = p_w1.tile([128, H], MM_DT, tag="w1", name=f"w1t_{s}_{m}")
                    nc.gpsimd.dma_start(out=w1t[:], in_=w1_d[s][m])
                pss = [p_ps.tile([128, tcz], F32, tag="ps", name=f"ps1_{s}_{m}_{ci}")
                       for ci, tcz in enumerate(v["tcs"])]
                C = CS[s]
                for k in range(KH):
                    for ci in range(len(v["tcs"])):
                        nc.tensor.matmul(
                            pss[ci][:], w1t[:, k * 128:(k + 1) * 128],
                            v["xt_all"][:, k * C + v["tco"][ci]:k * C + v["tco"][ci + 1]],
                            start=(k == 0), stop=(k == KH - 1))
                for ci in range(len(v["tcs"])):
                    nc.scalar.activation(
                        h1s(v, s, m, v["tco"][ci], v["tco"][ci + 1]), pss[ci][:],
                        mybir.ActivationFunctionType.Relu,
                        **({} if prescaled else {"bias": v["b1t"][m][:, 0:1]}))

            pre_w2 = {}

            def st2_group(s, m):
                v = st[s]
                if s == 0 and m in pre_w2:
                    w2t = pre_w2[m]
                else:
                    w2t = p_w2.tile([128, M], MM_DT, tag="w2", name=f"w2t_{s}_{m}")
                    nc.sync.dma_start(out=w2t[:], in_=w2_d[s][m])
                pss = [p_ps.tile([128, tcz], F32, tag="ps", name=f"ps2_{s}_{m}_{ci}")
                       for ci, tcz in enumerate(v["tcs"])]
                for k in range(KM):
                    for ci in range(len(v["tcs"])):
                        nc.tensor.matmul(
                            pss[ci][:], w2t[:, k * 128:(k + 1) * 128],
                            h1s(v, s, k, v["tco"][ci], v["tco"][ci + 1]),
                            start=(k == 0), stop=(k == KM - 1))
                for ci in range(len(v["tcs"])):
                    nc.scalar.activation(
                        h2s(v, s, m, v["tco"][ci], v["tco"][ci + 1]), pss[ci][:],
                        mybir.ActivationFunctionType.Relu,
                        **({} if prescaled else {"bias": v["b2t"][m][:, 0:1]}))

            def emit_w3(s, hb):
                w3t = [p_w3.tile([128, HB], MM_DT, tag=f"w3_{k % 4}", name=f"w3t_{s}_{hb}_{k}") for k in range(KM)]
                for k in range(KM):
                    nc.gpsimd.dma_start(
                        out=w3t[k][:],
                        in_=w3_d[s][k * 128:(k + 1) * 128, hb * HB:(hb + 1) * HB])
                st[s][f"w3_{hb}"] = w3t

            def st3_group(s, hb, t):
                v = st[s]
                t0, tn = v["tts"][t]
                w3t = v[f"w3_{hb}"]
                ps = p_ps.tile([tn, HB], F32, tag="ps", name=f"ps3_{s}_{hb}_{t}")
                for k in range(KM):
                    nc.tensor.matmul(
                        ps[:], h2s(v, s, k, t0, t0 + tn), w3t[k][:],
                        start=(k == 0), stop=(k == KM - 1))
                yt = p_y.tile([tn, HB], F32, tag="y", name=f"yt_{s}_{hb}_{t}")
                nc.scalar.activation(
                    yt[:], ps[:], mybir.ActivationFunctionType.Copy,
                    scale=v["cwt"][t][:, 0:1])
                nc.scalar.dma_start(
                    out=y_d[s][t0:t0 + tn, hb * HB:(hb + 1) * HB],
                    in_=yt[:])

            def st3_group_ws(s, hm):
                """Weight-stationary stage 3 (prescaled mode): out y_T[h, tok]."""
                v = st[s]
                w3t = p_w3.tile([128, M], MM_DT, tag="w3ws", name=f"w3ws_{s}_{hm}")
                nc.scalar.dma_start(out=w3t[:], in_=w3_d[s][hm])
                pss = [p_ps.tile([128, tcz], F32, tag="ps", name=f"ps3_{s}_{hm}_{ci}")
                       for ci, tcz in enumerate(v["tcs"])]
                for k in range(KM):
                    for ci in range(len(v["tcs"])):
                        nc.tensor.matmul(
                            pss[ci][:], w3t[:, k * 128:(k + 1) * 128],
                            h2s(v, s, k, v["tco"][ci], v["tco"][ci + 1]),
                            start=(k == 0), stop=(k == KM - 1))
                for ci, tcz in enumerate(v["tcs"]):
                    yt = p_y.tile([128, tcz], BF16, tag="y", name=f"yt_{s}_{hm}_{ci}")
                    nc.vector.tensor_scalar_mul(yt[:], pss[ci][:], 1.0)
                    nc.scalar.dma_start(
                        out=y_d[s][hm * 128:(hm + 1) * 128, v["tco"][ci]:v["tco"][ci + 1]],
                        in_=yt[:])

            def st3_emitters(s):
                if prescaled:
                    return [lambda s=s, hm=hm: st3_group_ws(s, hm) for hm in range(KH)]
                ems = []
                for hb in range(H // HB):
                    if hb > 0:
                        ems.append(lambda s=s, hb=hb: emit_w3(s, hb))
                    for t in range(len(st[s]["tts"])):
                        ems.append(lambda s=s, hb=hb, t=t: st3_group(s, hb, t))
                return ems

            def interleave(a_ems, b_ems):
                """Emit a and b emitter lists merged evenly (b spread among a)."""
                na, nb = len(a_ems), len(b_ems)
                bi = 0
                for i, a in enumerate(a_ems):
                    while bi < nb and bi * na <= i * nb:
                        b_ems[bi]()
                        bi += 1
                    a()
                while bi < nb:
                    b_ems[bi]()
                    bi += 1

            # ---- prime all three DMA rings with tiny transfers so the
            # ring-setup latency (1.6-4us) is paid during the boot barriers,
            # not in front of the first real weight loads ----
            for qi, eng in enumerate((nc.gpsimd, nc.sync, nc.scalar)):
                dt = p_b.tile([1, 64], MM_DT, tag=f"prime{qi}", name=f"prime{qi}")
                eng.dma_start(out=dt[:], in_=w1_d[0][0][0:1, 0:64])

            # ---- head of the gpsimd DMA queue: slot 0's first w1 tiles so
            # the PE can start the moment warmup ends ----
            for m in range(8):
                w1t = p_w1.tile([128, H], MM_DT, tag="w1", name=f"w1pre_{m}")
                nc.sync.dma_start(out=w1t[:], in_=w1_d[0][m])
                pre_w1[m] = w1t

            # ---- PE pre-warm: ~4us of junk matmuls so the HAM clock-gate
            # reaches 2.4 GHz while the first input DMAs land ----
            junk = p_b.tile([128, 512], MM_DT, tag="warm", name="warm_src")
            nc.vector.memset(junk[:], 0.0)
            psw = p_ps.tile([128, 512], F32, tag="ps", name="warm_ps")
            for i in range(10):
                nc.tensor.matmul(psw[:], junk[:, 0:128], junk[:],
                                 start=(i == 0), stop=(i == 9))

            # ---- emission schedule: st3(s-1) interleaves with st1(s) ----
            emit_loads(0)
            # prefetch slot-0 stage-2 weights on the scalar queue (idle at
            # startup) so st2(0) isn't gated by the sync queue's ramp
            for m in range(8):
                w2t = p_w2.tile([128, M], MM_DT, tag="w2", name=f"w2pre_{m}")
                nc.scalar.dma_start(out=w2t[:], in_=w2_d[0][m])
                pre_w2[m] = w2t
            prev_st3 = []
            for s in range(S):
                if s > 0:
                    emit_loads(s)
                interleave([lambda s=s, m=m: st1_group(s, m) for m in range(KM)],
                           prev_st3)
                if not prescaled:
                    emit_w3(s, 0)    # prefetch stage-3 hb=0 weights early
                for m in range(KM):
                    st2_group(s, m)
                prev_st3 = st3_emitters(s)
            for em in prev_st3:
                em()

    if legalize:
        _legalize_waits(nc)
    return nc


# ---------------------------------------------------------------------------
def _run_spmd(CS, prescaled, in_maps):
    """Compile + run on cores 0-7. On a transient device failure (e.g.
    NRT_EXEC_UNIT_UNRECOVERABLE from a stale runtime state), retry in a
    fresh subprocess whose NRT session starts clean."""
    try:
        nc = _build_program(CS, prescaled=prescaled)
        return run_bass_kernel_spmd(nc, in_maps, list(range(N_CORES))).results
    except Exception:
        import os
        import pickle
        import subprocess
        import sys
        import tempfile
        d = tempfile.mkdtemp()
        inp, outp = os.path.join(d, "in.pkl"), os.path.join(d, "out.pkl")
        with open(inp, "wb") as f:
            pickle.dump((CS, prescaled, in_maps), f)
        code = (
            "import pickle, sys\n"
            f"sys.path.insert(0, {os.path.dirname(os.path.abspath(__file__))!r})\n"
            "import kernel as K\n"
            f"CS, prescaled, in_maps = pickle.load(open({inp!r}, 'rb'))\n"
            "nc = K._build_program(CS, prescaled=prescaled)\n"
            "from concourse.bass_utils import run_bass_kernel_spmd\n"
            "r = run_bass_kernel_spmd(nc, in_maps, list(range(K.N_CORES))).results\n"
            f"pickle.dump(r, open({outp!r}, 'wb'))\n"
        )
        err = None
        for _ in range(2):
            try:
                subprocess.run([sys.executable, "-c", code], check=True,
                               timeout=1800)
                with open(outp, "rb") as f:
                    return pickle.load(f)
            except Exception as e:
                err = e
        raise err


def kernel(x, genre_embed, rms_w, wg_W, wg_b, gg_W, gg_b, W1, b1, W2, b2, W3, b3):
    x = np.asarray(x, np.float32)
    B, S_, _ = x.shape
    T = B * S_
    x2d = np.ascontiguousarray(x.reshape(T, H))
    W1 = np.asarray(W1, np.float32)
    W2 = np.asarray(W2, np.float32)
    W3 = np.asarray(W3, np.float32)

    if MM_DT == BF16:
        import ml_dtypes
        host_dt = ml_dtypes.bfloat16
    else:
        host_dt = np.float32

    cw = _route(x2d, np.asarray(genre_embed, np.float32), np.asarray(rms_w, np.float32),
                np.asarray(wg_W, np.float32), np.asarray(wg_b, np.float32),
                np.asarray(gg_W, np.float32), np.asarray(gg_b, np.float32), B, S_)
    E = cw.shape[1]
    tok_by_e = [np.nonzero(cw[:, e])[0] for e in range(E)]
    counts = [len(t) for t in tok_by_e]
    CS, slots = _partition(counts)

    # prescaled mode is exact when the in-MLP biases are zero (cw >= 0
    # commutes with relu); b3 is always applied on the host via cw @ b3
    prescaled = not (np.any(np.asarray(b1)) or np.any(np.asarray(b2)))

    # pre-tile weights once per expert (shared across cores)
    used = set(s[0] for core in slots for s in core if s)
    w1_tiled, w2_tiled, w3_tiled = {}, {}, {}
    for e in used:
        w1_tiled[e] = np.ascontiguousarray(
            W1[e].reshape(KH, 128, KM, 128).transpose(2, 1, 0, 3).reshape(KM, 128, H)).astype(host_dt)
        w2_tiled[e] = np.ascontiguousarray(
            W2[e].reshape(KM, 128, KM, 128).transpose(2, 1, 0, 3).reshape(KM, 128, M)).astype(host_dt)
        if prescaled:
            w3_tiled[e] = np.ascontiguousarray(
                W3[e].reshape(KM, 128, KH, 128).transpose(2, 1, 0, 3).reshape(KH, 128, M)).astype(host_dt)
        else:
            w3_tiled[e] = W3[e].astype(host_dt)

    e_any = next(iter(used))
    in_maps = []
    meta = []
    for core in range(N_CORES):
        im = {}
        cmeta = []
        for si in range(len(CS)):
            C = CS[si]
            slot = slots[core][si]
            e, lo, hi = slot if slot else (e_any, 0, 0)
            idx = tok_by_e[e][lo:hi]
            n = len(idx)
            xt = np.zeros((H, C), host_dt)
            if prescaled:
                xt[:, :n] = (x2d[idx] * cw[idx, e][:, None]).T.astype(host_dt)
            else:
                xt[:, :n] = x2d[idx].T.astype(host_dt)
            im[f"XT{si}"] = np.ascontiguousarray(
                xt.reshape(KH, 128, C).transpose(1, 0, 2).reshape(128, KH * C))
            im[f"W1{si}"] = w1_tiled[e]
            im[f"W2{si}"] = w2_tiled[e]
            im[f"W3{si}"] = w3_tiled[e]
            if not prescaled:
                cwc = np.zeros((C,), np.float32)
                cwc[:n] = cw[idx, e]
                im[f"B1{si}"] = np.asarray(b1[e], np.float32)
                im[f"B2{si}"] = np.asarray(b2[e], np.float32)
                im[f"CW{si}"] = cwc
            cmeta.append(idx)
        in_maps.append(im)
        meta.append(cmeta)

    results = _run_spmd(CS, prescaled, in_maps)

    out2d = cw @ np.asarray(b3, np.float32)      # bias-3 combine term [T, H]
    for core in range(N_CORES):
        for si, idx in enumerate(meta[core]):
            y = results[core][f"Y{si}"]
            if prescaled:
                out2d[idx] += np.asarray(y[:, :len(idx)], np.float32).T
            else:
                out2d[idx] += y[:len(idx)]
    return out2d.reshape(B, S_, H).astype(np.float32)

